# revision 34
# baseline (speedup 1.0000x reference)
"""Multi-head attention (B=2, T=4096, D=512, H=8) on 8 TRN2 NeuronCores.

Sharding: core c handles batch c//4 and query rows (c%4)*1024..+1024.
Heads stay together on a core; K/V are recomputed per core (no comm).

v2 layout: single fused stream — the K/Q/V projection tiles are emitted
just-in-time inside the attention round loop so the ScalarE exp stream
(the critical engine, ~1 elem/lane/cycle over 33.5M elems/core) starts
a few microseconds in instead of after the whole projection phase.
ScalarE runs exp only; every PSUM->SBUF copy and the softmax
normalization run on VectorE (reciprocal_approx_fast reads the
denominator row straight from PSUM; one 2-row selector matmul
broadcasts both heads' reciprocals across partitions).

Per-round dataflow (jq query half, pr head pair, kc key chunk):
  S^T pair = kt[0:64].T@qt[0:64] | kt[64:128].T@qt[64:128]  (row-tiled,
  concurrent 64-row PE tiles) -> exp(0.125 x) on ScalarE -> mask01
  multiply on VectorE (bf16, 2x mode) -> O^T(+denominator row) +=
  V_aug.T @ P accumulated over all 32 kc in PSUM.
"""

import sys

sys.path.insert(0, "/opt/trn_rl_repo")

import numpy as np
import ml_dtypes

B, T, D, H = 2, 4096, 512, 8
DH = D // H          # 64
N_CORES = 8
QPC = 1024           # query rows per core
DC = D // 128        # 4 partition chunks of the model dim
KC = T // 128        # 32 key chunks
NMB = 9              # rotating mask buffers of [128, 4kc, 512q]

_BUILT = {}


def _build(with_bias: bool):
    from concourse import bacc
    import concourse.mybir as mybir
    import concourse.tile as tile

    dt = mybir.dt
    AF = mybir.ActivationFunctionType

    nc = bacc.Bacc("TRN2", target_bir_lowering=False, debug=False,
                   num_devices=N_CORES)

    xT = nc.dram_tensor("xT", [128, DC, T], dt.bfloat16, kind="ExternalInput").ap()
    xQT = nc.dram_tensor("xQT", [128, DC, QPC], dt.bfloat16, kind="ExternalInput").ap()
    wq = nc.dram_tensor("wq", [128, DC, D], dt.bfloat16, kind="ExternalInput").ap()
    wk = nc.dram_tensor("wk", [128, DC, D], dt.bfloat16, kind="ExternalInput").ap()
    wv = nc.dram_tensor("wv", [128, DC, D], dt.bfloat16, kind="ExternalInput").ap()
    wo = nc.dram_tensor("wo", [128, DC, D], dt.bfloat16, kind="ExternalInput").ap()
    # mask chunks: index (jq*8+g) -> [128, 4 kc, 512 q] as 0/1 bf16
    msk = nc.dram_tensor("msk", [16, 128, 4, 512], dt.bfloat16, kind="ExternalInput").ap()
    sel = nc.dram_tensor("sel", [2, 128], dt.float16, kind="ExternalInput").ap()
    if with_bias:
        bqkd = nc.dram_tensor("bqk", [128, DC, 2], dt.float32, kind="ExternalInput").ap()
        bvo = nc.dram_tensor("bvo", [1, 2, D], dt.bfloat16, kind="ExternalInput").ap()
        ones1 = nc.dram_tensor("ones1", [1, 128], dt.bfloat16, kind="ExternalInput").ap()
    out = nc.dram_tensor("out", [QPC, D], dt.float32, kind="ExternalOutput").ap()

    with tile.TileContext(nc) as tc:
        with (
            tc.tile_pool(name="persist", bufs=1) as pp,
            tc.tile_pool(name="psS", bufs=2, space="PSUM") as psS,
            tc.tile_pool(name="psO", bufs=1, space="PSUM") as psO,
            tc.tile_pool(name="psX", bufs=2, space="PSUM") as psX,
            tc.tile_pool(name="pP", bufs=8) as pP,
            tc.tile_pool(name="pN", bufs=1) as pN,
            tc.tile_pool(name="pC", bufs=2) as pC,
        ):
            wq_sb = pp.tile([128, DC, D], dt.bfloat16, tag="wq")
            wk_sb = pp.tile([128, DC, D], dt.bfloat16, tag="wk")
            wv_sb = pp.tile([128, DC, D], dt.bfloat16, tag="wv")
            wo_sb = pp.tile([128, DC, D], dt.bfloat16, tag="wo")
            sel_sb = pp.tile([2, 128], dt.float16, tag="sel")
            xt_sb = pp.tile([128, DC, T], dt.bfloat16, tag="xt")
            xqt_sb = pp.tile([128, DC, QPC], dt.bfloat16, tag="xqt")
            kt_f = [pp.tile([128, T], dt.bfloat16, tag=f"kt{fo}", name=f"kt{fo}")
                    for fo in range(DC)]
            v_g = [pp.tile([128, KC // 4, H, DH + 1], dt.bfloat16, tag=f"v{g}", name=f"v{g}")
                   for g in range(4)]
            qt_f = [pp.tile([128, QPC], dt.bfloat16, tag=f"qt{fo}", name=f"qt{fo}")
                    for fo in range(DC)]
            ot_f = [pp.tile([128, QPC], dt.bfloat16, tag=f"ot{fo}", name=f"ot{fo}")
                    for fo in range(DC)]
            mt = [pp.tile([128, 4, 512], dt.bfloat16, tag=f"m{i}", name=f"m{i}")
                  for i in range(NMB)]

            # ---- DMA issue order = arrival order ----
            # smallest possible prefix for round 0: fo0 weight slices +
            # first x^T chunk, then everything else
            nc.sync.dma_start(wk_sb[:, :, 0:128], wk[:, :, 0:128])
            nc.sync.dma_start(wq_sb[:, :, 0:128], wq[:, :, 0:128])
            nc.sync.dma_start(xt_sb[:, :, 0:512], xT[:, :, 0:512])
            nc.sync.dma_start(xqt_sb[:, :, 0:512], xQT[:, :, 0:512])
            nc.sync.dma_start(mt[0][:], msk[0])
            nc.sync.dma_start(wv_sb[:], wv[:])
            nc.sync.dma_start(sel_sb[:], sel[:])
            nc.sync.dma_start(xqt_sb[:, :, 512:1024], xQT[:, :, 512:1024])
            nc.sync.dma_start(wk_sb[:, :, 128:512], wk[:, :, 128:512])
            nc.sync.dma_start(wq_sb[:, :, 128:512], wq[:, :, 128:512])
            for nb in range(1, T // 512):
                nc.sync.dma_start(xt_sb[:, :, nb * 512:(nb + 1) * 512],
                                  xT[:, :, nb * 512:(nb + 1) * 512])
            if with_bias:
                bqk_sb = pp.tile([128, DC, 2], dt.float32, tag="bqk")
                bvo_sb = pp.tile([1, 2, D], dt.bfloat16, tag="bvo")
                ones1_sb = pp.tile([1, 128], dt.bfloat16, tag="ones1")
                nc.sync.dma_start(bqk_sb[:], bqkd[:])
                nc.sync.dma_start(bvo_sb[:], bvo[:])
                nc.sync.dma_start(ones1_sb[:], ones1[:])
            for i in range(1, NMB):        # jq0 masks + jq1 group 0
                nc.sync.dma_start(mt[i][:], msk[i])
            nc.sync.dma_start(wo_sb[:], wo[:])
            # ones column of V_aug (denominator accumulator)
            for g in range(4):
                nc.vector.memset(v_g[g][:, :, :, DH:DH + 1], 1.0)
            # warm the ACT exp table set during the DMA wait (~2.7us
            # table load that would otherwise delay the first real exp)
            warm = pN.tile([1, 8], dt.float32, tag="warm")
            nc.vector.memset(warm[:], 0.0)
            nc.scalar.activation(warm[:], warm[:], AF.Exp, scale=1.0)

            # ---- projection tile emitters (PSUM via psX, copies on DVE) ----
            def kq_tile(w_sb, src_sb, out_ap, nb, bi, fo):
                ps = psX.tile([128, 512], dt.float32, tag="psX")
                for dc in range(DC):
                    nc.tensor.matmul(
                        ps[:],
                        w_sb[:, dc, fo * 128:(fo + 1) * 128],
                        src_sb[:, dc, nb * 512:(nb + 1) * 512],
                        start=(dc == 0), stop=(dc == DC - 1),
                    )
                if with_bias:
                    nc.vector.tensor_scalar(
                        out_ap, ps[:], bqk_sb[:, fo, bi:bi + 1], 0.0,
                        mybir.AluOpType.add, mybir.AluOpType.bypass,
                    )
                else:
                    nc.vector.tensor_copy(out_ap, ps[:])

            def k_tile(fo, nb):
                kq_tile(wk_sb, xt_sb, kt_f[fo][:, nb * 512:(nb + 1) * 512], nb, 1, fo)

            def q_tile(fo, nb):
                kq_tile(wq_sb, xqt_sb, qt_f[fo][:, nb * 512:(nb + 1) * 512], nb, 0, fo)

            def v_tile(tt):
                ps = psX.tile([128, 512], dt.float32, tag="psX")
                for dc in range(DC):
                    nc.tensor.matmul(
                        ps[:],
                        xt_sb[:, dc, tt * 128:(tt + 1) * 128],
                        wv_sb[:, dc, :],
                        start=(dc == 0),
                        stop=(not with_bias and dc == DC - 1),
                    )
                if with_bias:
                    nc.tensor.matmul(ps[:], ones1_sb[:], bvo_sb[:, 0, :],
                                     start=False, stop=True)
                nc.vector.tensor_copy(
                    v_g[tt // 8][:, tt % 8, :, 0:DH],
                    ps[:].rearrange("p (h f) -> p h f", h=H))

            # just-in-time emission bookkeeping
            emitted = set()

            def need(item):
                if item in emitted:
                    return
                emitted.add(item)
                kind, a, b = item
                if kind == "k":
                    k_tile(a, b)
                elif kind == "q":
                    q_tile(a, b)
                else:
                    v_tile(a)

            # background queue: projection work not strictly required yet,
            # drained at a bounded rate during rounds so later pairs'
            # inputs are ready ahead of their first use.
            bg = []
            for fo in range(1, DC):
                bg.append(("q", fo, 0))
                for nb in range(T // 512):
                    bg.append(("k", fo, nb))
                bg.append(("q", fo, 1))
            bg.insert(12, ("q", 0, 1))
            bgi = [0]

            def drain_bg(n):
                while n > 0 and bgi[0] < len(bg):
                    need(bg[bgi[0]])
                    bgi[0] += 1
                    n -= 1

            def norm_pair(ot_e, ot_o, pr, qs):
                # stage 1 (all DVE): reciprocal of both denominator rows,
                # the two heads' chains interleaved to hide dep latency
                den = {}
                rech = {}
                for i, ot_ps in ((0, ot_e), (1, ot_o)):
                    den[i] = pN.tile([1, 512], dt.float32, tag=f"den{i}",
                                     name=f"den{i}")
                    nc.vector.tensor_copy(den[i][:], ot_ps[DH:DH + 1, :])
                rec = {}
                for i in (0, 1):
                    rec[i] = pN.tile([1, 512], dt.float32, tag=f"rec{i}",
                                     name=f"rec{i}")
                    nc.vector.reciprocal_approx_fast(rec[i][:], den[i][:])
                for i in (0, 1):
                    rech[i] = pN.tile([1, 512], dt.float16, tag=f"rech{i}",
                                      name=f"rech{i}")
                    nc.vector.tensor_copy(rech[i][:], rec[i][:])

                def stage2(i, ot_ps):
                    # broadcast across 64 partitions + apply
                    bc = psX.tile([64, 512], dt.float32, tag="psX")
                    nc.tensor.matmul(bc[:], sel_sb[0:1, 0:64], rech[i][:],
                                     start=True, stop=True)
                    bcs = pN.tile([64, 512], dt.float32, tag=f"bcs{i}")
                    nc.vector.tensor_copy(bcs[:], bc[:])
                    nc.vector.tensor_mul(
                        ot_f[pr][i * 64:(i + 1) * 64, qs],
                        ot_ps[0:DH, :], bcs[:])
                return (lambda: stage2(0, ot_e)), (lambda: stage2(1, ot_o))

            def out_proj_tile(tt):
                ps = psX.tile([128, 512], dt.float32, tag="psX")
                for dc in range(DC):
                    nc.tensor.matmul(
                        ps[:],
                        ot_f[dc][:, tt * 128:(tt + 1) * 128],
                        wo_sb[:, dc, :],
                        start=(dc == 0),
                        stop=(with_bias is False and dc == DC - 1))
                if with_bias:
                    nc.tensor.matmul(ps[:], ones1_sb[:],
                                     bvo_sb[:, 1, :],
                                     start=False, stop=True)
                os = pC.tile([128, 512], dt.float32, tag="os")
                nc.vector.tensor_copy(os[:], ps[:])
                nc.sync.dma_start(out[tt * 128:(tt + 1) * 128, :], os[:])

            # ---- fused attention rounds: jq outer, pr mid, kc inner.
            # PV lags scores by 2 rounds; the previous sweep's
            # normalization and the finished half's output projection are
            # deferred into the first rounds of the following sweep so
            # they never head-of-line-block the PE queue.
            deferred = []      # norm stage-2 closures, drained rounds 2-3
            def_op = []        # out-proj closures, drained rounds 8..14
            for jq in range(2):
                qs = slice(jq * 512, (jq + 1) * 512)
                for pr in range(H // 2):
                    need(("q", pr, jq))
                    ot_e = psO.tile([DH + 1, 512], dt.float32, tag="ote")
                    ot_o = psO.tile([DH + 1, 512], dt.float32, tag="oto")
                    pvq = []
                    for kc in range(KC):
                        need(("k", pr, kc // 4))
                        if pr == 0 and jq == 0:
                            need(("v", kc, 0))
                            if kc >= 16 and kc % 2 == 1:
                                drain_bg(1)
                        elif jq == 0 and kc % 2 == 0:
                            drain_bg(1)
                        ks = slice(kc * 128, (kc + 1) * 128)
                        sp = psS.tile([128, 1024], dt.float32, tag="sp")
                        nc.tensor.matmul(
                            sp[:, 0:512],
                            kt_f[pr][0:64, ks], qt_f[pr][0:64, qs],
                            start=True, stop=True)
                        nc.tensor.matmul(
                            sp[:, 512:1024],
                            kt_f[pr][64:128, ks], qt_f[pr][64:128, qs],
                            start=True, stop=True)
                        p_sb = pP.tile([128, 1024], dt.bfloat16, tag="p")
                        nc.scalar.activation(p_sb[:], sp[:], AF.Exp,
                                             scale=0.125)
                        mb = mt[(jq * 8 + kc // 4) % NMB]
                        pv = p_sb[:].rearrange("p (a b) -> p a b", a=2)
                        nc.vector.tensor_mul(
                            pv, pv,
                            mb[:, kc % 4, :][:, None, :]
                            .to_broadcast((128, 2, 512)))

                        def pv_mm(kc=kc, p_sb=p_sb):
                            nc.tensor.matmul(
                                ot_e[:], v_g[kc // 8][:, kc % 8, 2 * pr, :],
                                p_sb[:, 0:512],
                                start=(kc == 0), stop=(kc == KC - 1))
                            nc.tensor.matmul(
                                ot_o[:], v_g[kc // 8][:, kc % 8, 2 * pr + 1, :],
                                p_sb[:, 512:1024],
                                start=(kc == 0), stop=(kc == KC - 1))
                        pvq.append(pv_mm)
                        if deferred and kc in (2, 3):
                            deferred.pop(0)()
                        if def_op and 4 <= kc < 28 and kc % 2 == 0:
                            def_op.pop(0)()
                        lag = 4 if kc < 8 else (2 if kc < 30 else 31 - kc)
                        while len(pvq) > lag:
                            pvq.pop(0)()
                        if jq == 0 and pr == 3 and kc % 4 == 3 and kc // 4 >= 1:
                            # refill mask buffer with jq1's next group once
                            # its last jq0 reader (this round) is done
                            nc.sync.dma_start(mt[(8 + kc // 4) % NMB][:],
                                              msk[8 + kc // 4])
                    while pvq:
                        pvq.pop(0)()
                    assert not deferred
                    # queue this sweep's normalization for the next sweep
                    s2e, s2o = norm_pair(ot_e, ot_o, pr, qs)
                    deferred.append(s2e)
                    deferred.append(s2o)
                    if pr == H // 2 - 1:
                        for tt in range(jq * 4, (jq + 1) * 4):
                            def_op.append(lambda tt=tt: out_proj_tile(tt))
            while deferred:
                deferred.pop(0)()
            while def_op:
                def_op.pop(0)()

    nc.compile()
    return nc


def _get_nc(with_bias: bool):
    if with_bias not in _BUILT:
        _BUILT[with_bias] = _build(with_bias)
    return _BUILT[with_bias]


def _prep_inputs(x, Wq, bq, Wk, bk, Wv, bv, Wo, bo, mask, with_bias):
    bf16 = ml_dtypes.bfloat16

    shared = {}
    for name, W in (("wq", Wq), ("wk", Wk), ("wv", Wv), ("wo", Wo)):
        shared[name] = np.ascontiguousarray(
            np.asarray(W, np.float32).astype(bf16)
            .reshape(DC, 128, D).transpose(1, 0, 2))
    selm = np.zeros((2, 128), np.float32)
    selm[0, 0:64] = 1.0
    selm[1, 64:128] = 1.0
    shared["sel"] = selm.astype(np.float16)
    if with_bias:
        shared["bqk"] = np.ascontiguousarray(np.stack(
            [np.asarray(bq, np.float32).reshape(DC, 128).T,
             np.asarray(bk, np.float32).reshape(DC, 128).T], axis=-1))
        shared["bvo"] = np.ascontiguousarray(np.stack(
            [np.asarray(bv, np.float32), np.asarray(bo, np.float32)]
        ).astype(bf16).reshape(1, 2, D))
        shared["ones1"] = np.ones((1, 128), np.float32).astype(bf16)

    maskT = np.asarray(mask).reshape(T, T).T          # (k, q)
    m01T = maskT.astype(np.float32)

    in_maps = []
    for c in range(N_CORES):
        b, qlo = c // 4, (c % 4) * QPC
        xTb = np.asarray(x[b], np.float32).T.astype(bf16)     # (D, T)
        m = dict(shared)
        m["xT"] = np.ascontiguousarray(
            xTb.reshape(DC, 128, T).transpose(1, 0, 2))
        m["xQT"] = np.ascontiguousarray(
            xTb[:, qlo:qlo + QPC].reshape(DC, 128, QPC).transpose(1, 0, 2))
        # (k, q) block of this core's queries -> [jq, g, 128, 4, 512]
        m01 = m01T[:, qlo:qlo + QPC].reshape(8, 4, 128, 2, 512)
        m["msk"] = np.ascontiguousarray(
            m01.transpose(3, 0, 2, 1, 4).reshape(16, 128, 4, 512)).astype(bf16)
        in_maps.append(m)
    return in_maps


def _run(inputs, trace=False):
    from concourse.bass_utils import run_bass_kernel_spmd

    with_bias = any(
        float(np.abs(np.asarray(inputs[k], np.float32)).max()) != 0.0
        for k in ("bq", "bk", "bv", "bo"))
    nc = _get_nc(with_bias)
    in_maps = _prep_inputs(
        inputs["x"], inputs["Wq"], inputs["bq"], inputs["Wk"], inputs["bk"],
        inputs["Wv"], inputs["bv"], inputs["Wo"], inputs["bo"],
        inputs["mask"], with_bias)
    res = run_bass_kernel_spmd(nc, in_maps, list(range(N_CORES)), trace=trace)
    O = np.empty((B, T, D), np.float32)
    for c in range(N_CORES):
        b, qlo = c // 4, (c % 4) * QPC
        O[b, qlo:qlo + QPC, :] = res.results[c]["out"]
    return O, res


def kernel(**inputs) -> np.ndarray:
    out, _ = _run(inputs, trace=False)
    return out


# revision 37
# speedup vs baseline: 1.0209x; 1.0209x over previous
"""Multi-head attention (B=2, T=4096, D=512, H=8) on 8 TRN2 NeuronCores.

Sharding: core c handles batch c//4 and query rows (c%4)*1024..+1024.
Heads stay together on a core; K/V are recomputed per core (no comm).

v2 layout: single fused stream — the K/Q/V projection tiles are emitted
just-in-time inside the attention round loop so the ScalarE exp stream
(the critical engine, ~1 elem/lane/cycle over 33.5M elems/core) starts
a few microseconds in instead of after the whole projection phase.
ScalarE runs exp only; every PSUM->SBUF copy and the softmax
normalization run on VectorE (reciprocal_approx_fast reads the
denominator row straight from PSUM; one 2-row selector matmul
broadcasts both heads' reciprocals across partitions).

Per-round dataflow (jq query half, pr head pair, kc key chunk):
  S^T pair = kt[0:64].T@qt[0:64] | kt[64:128].T@qt[64:128]  (row-tiled,
  concurrent 64-row PE tiles) -> exp(0.125 x) on ScalarE -> mask01
  multiply on VectorE (bf16, 2x mode) -> O^T(+denominator row) +=
  V_aug.T @ P accumulated over all 32 kc in PSUM.
"""

import sys

sys.path.insert(0, "/opt/trn_rl_repo")

import numpy as np
import ml_dtypes

B, T, D, H = 2, 4096, 512, 8
DH = D // H          # 64
N_CORES = 8
QPC = 1024           # query rows per core
DC = D // 128        # 4 partition chunks of the model dim
KC = T // 128        # 32 key chunks
NMB = 9              # rotating mask buffers of [128, 4kc, 512q]

_BUILT = {}


def _build(with_bias: bool):
    from concourse import bacc
    import concourse.mybir as mybir
    import concourse.tile as tile

    dt = mybir.dt
    AF = mybir.ActivationFunctionType

    nc = bacc.Bacc("TRN2", target_bir_lowering=False, debug=False,
                   num_devices=N_CORES)

    xT = nc.dram_tensor("xT", [128, DC, T], dt.bfloat16, kind="ExternalInput").ap()
    xQT = nc.dram_tensor("xQT", [128, DC, QPC], dt.bfloat16, kind="ExternalInput").ap()
    wq = nc.dram_tensor("wq", [128, DC, D], dt.bfloat16, kind="ExternalInput").ap()
    wk = nc.dram_tensor("wk", [128, DC, D], dt.bfloat16, kind="ExternalInput").ap()
    wv = nc.dram_tensor("wv", [128, DC, D], dt.bfloat16, kind="ExternalInput").ap()
    wo = nc.dram_tensor("wo", [128, DC, D], dt.bfloat16, kind="ExternalInput").ap()
    # mask chunks: index (jq*8+g) -> [128, 4 kc, 512 q] as 0/1 bf16
    msk = nc.dram_tensor("msk", [16, 128, 4, 512], dt.bfloat16, kind="ExternalInput").ap()
    sel = nc.dram_tensor("sel", [2, 128], dt.float16, kind="ExternalInput").ap()
    if with_bias:
        bqkd = nc.dram_tensor("bqk", [128, DC, 2], dt.float32, kind="ExternalInput").ap()
        bvo = nc.dram_tensor("bvo", [1, 2, D], dt.bfloat16, kind="ExternalInput").ap()
        ones1 = nc.dram_tensor("ones1", [1, 128], dt.bfloat16, kind="ExternalInput").ap()
    out = nc.dram_tensor("out", [QPC, D], dt.float32, kind="ExternalOutput").ap()

    with tile.TileContext(nc) as tc:
        with (
            tc.tile_pool(name="persist", bufs=1) as pp,
            tc.tile_pool(name="psS", bufs=2, space="PSUM") as psS,
            tc.tile_pool(name="psO", bufs=1, space="PSUM") as psO,
            tc.tile_pool(name="psX", bufs=2, space="PSUM") as psX,
            tc.tile_pool(name="pP", bufs=8) as pP,
            tc.tile_pool(name="pN", bufs=1) as pN,
            tc.tile_pool(name="pC", bufs=2) as pC,
        ):
            wq_sb = pp.tile([128, DC, D], dt.bfloat16, tag="wq")
            wk_sb = pp.tile([128, DC, D], dt.bfloat16, tag="wk")
            wv_sb = pp.tile([128, DC, D], dt.bfloat16, tag="wv")
            wo_sb = pp.tile([128, DC, D], dt.bfloat16, tag="wo")
            sel_sb = pp.tile([2, 128], dt.float16, tag="sel")
            xt_sb = pp.tile([128, DC, T], dt.bfloat16, tag="xt")
            xqt_sb = pp.tile([128, DC, QPC], dt.bfloat16, tag="xqt")
            kt_f = [pp.tile([128, T], dt.bfloat16, tag=f"kt{fo}", name=f"kt{fo}")
                    for fo in range(DC)]
            v_g = [pp.tile([128, KC // 4, H, DH + 1], dt.bfloat16, tag=f"v{g}", name=f"v{g}")
                   for g in range(4)]
            qt_f = [pp.tile([128, QPC], dt.bfloat16, tag=f"qt{fo}", name=f"qt{fo}")
                    for fo in range(DC)]
            ot_f = [pp.tile([128, QPC], dt.bfloat16, tag=f"ot{fo}", name=f"ot{fo}")
                    for fo in range(DC)]
            mt = [pp.tile([128, 4, 512], dt.bfloat16, tag=f"m{i}", name=f"m{i}")
                  for i in range(NMB)]

            # ---- DMA issue order = arrival order ----
            # smallest possible prefix for round 0: fo0 weight slices +
            # first x^T chunk, then everything else
            nc.sync.dma_start(wk_sb[:, :, 0:128], wk[:, :, 0:128])
            nc.sync.dma_start(wq_sb[:, :, 0:128], wq[:, :, 0:128])
            nc.sync.dma_start(xt_sb[:, :, 0:512], xT[:, :, 0:512])
            nc.sync.dma_start(xqt_sb[:, :, 0:512], xQT[:, :, 0:512])
            nc.sync.dma_start(mt[0][:], msk[0])
            nc.sync.dma_start(wv_sb[:], wv[:])
            nc.sync.dma_start(sel_sb[:], sel[:])
            nc.sync.dma_start(xqt_sb[:, :, 512:1024], xQT[:, :, 512:1024])
            nc.sync.dma_start(wk_sb[:, :, 128:512], wk[:, :, 128:512])
            nc.sync.dma_start(wq_sb[:, :, 128:512], wq[:, :, 128:512])
            for nb in range(1, T // 512):
                nc.sync.dma_start(xt_sb[:, :, nb * 512:(nb + 1) * 512],
                                  xT[:, :, nb * 512:(nb + 1) * 512])
            if with_bias:
                bqk_sb = pp.tile([128, DC, 2], dt.float32, tag="bqk")
                bvo_sb = pp.tile([1, 2, D], dt.bfloat16, tag="bvo")
                ones1_sb = pp.tile([1, 128], dt.bfloat16, tag="ones1")
                nc.sync.dma_start(bqk_sb[:], bqkd[:])
                nc.sync.dma_start(bvo_sb[:], bvo[:])
                nc.sync.dma_start(ones1_sb[:], ones1[:])
            for i in range(1, NMB):        # jq0 masks + jq1 group 0
                nc.sync.dma_start(mt[i][:], msk[i])
            nc.sync.dma_start(wo_sb[:], wo[:])
            # ones column of V_aug (denominator accumulator)
            for g in range(4):
                nc.vector.memset(v_g[g][:, :, :, DH:DH + 1], 1.0)
            # warm the ACT exp table set during the DMA wait (~2.7us
            # table load that would otherwise delay the first real exp)
            warm = pN.tile([1, 8], dt.float32, tag="warm")
            nc.vector.memset(warm[:], 0.0)
            nc.scalar.activation(warm[:], warm[:], AF.Exp, scale=1.0)

            # ---- projection tile emitters (PSUM via psX, copies on DVE) ----
            def kq_tile(w_sb, src_sb, out_ap, nb, bi, fo):
                ps = psX.tile([128, 512], dt.float32, tag="psX")
                for dc in range(DC):
                    nc.tensor.matmul(
                        ps[:],
                        w_sb[:, dc, fo * 128:(fo + 1) * 128],
                        src_sb[:, dc, nb * 512:(nb + 1) * 512],
                        start=(dc == 0), stop=(dc == DC - 1),
                    )
                if with_bias:
                    nc.vector.tensor_scalar(
                        out_ap, ps[:], bqk_sb[:, fo, bi:bi + 1], 0.0,
                        mybir.AluOpType.add, mybir.AluOpType.bypass,
                    )
                else:
                    nc.vector.tensor_copy(out_ap, ps[:])

            def k_tile(fo, nb):
                kq_tile(wk_sb, xt_sb, kt_f[fo][:, nb * 512:(nb + 1) * 512], nb, 1, fo)

            def q_tile(fo, nb):
                kq_tile(wq_sb, xqt_sb, qt_f[fo][:, nb * 512:(nb + 1) * 512], nb, 0, fo)

            def v_tile(tt):
                ps = psX.tile([128, 512], dt.float32, tag="psX")
                for dc in range(DC):
                    nc.tensor.matmul(
                        ps[:],
                        xt_sb[:, dc, tt * 128:(tt + 1) * 128],
                        wv_sb[:, dc, :],
                        start=(dc == 0),
                        stop=(not with_bias and dc == DC - 1),
                    )
                if with_bias:
                    nc.tensor.matmul(ps[:], ones1_sb[:], bvo_sb[:, 0, :],
                                     start=False, stop=True)
                nc.vector.tensor_copy(
                    v_g[tt // 8][:, tt % 8, :, 0:DH],
                    ps[:].rearrange("p (h f) -> p h f", h=H))

            # just-in-time emission bookkeeping
            emitted = set()

            def need(item):
                if item in emitted:
                    return
                emitted.add(item)
                kind, a, b = item
                if kind == "k":
                    k_tile(a, b)
                elif kind == "q":
                    q_tile(a, b)
                else:
                    v_tile(a)

            # background queue: projection work not strictly required yet,
            # drained at a bounded rate during rounds so later pairs'
            # inputs are ready ahead of their first use.
            bg = []
            for fo in range(1, DC):
                bg.append(("q", fo, 0))
                for nb in range(T // 512):
                    bg.append(("k", fo, nb))
                bg.append(("q", fo, 1))
            bg.insert(12, ("q", 0, 1))
            bgi = [0]

            def drain_bg(n):
                while n > 0 and bgi[0] < len(bg):
                    need(bg[bgi[0]])
                    bgi[0] += 1
                    n -= 1

            def norm_pair(ot_e, ot_o, pr, qs):
                # stage 1 (all DVE): reciprocal of both denominator rows,
                # the two heads' chains interleaved to hide dep latency
                den = {}
                rech = {}
                for i, ot_ps in ((0, ot_e), (1, ot_o)):
                    den[i] = pN.tile([1, 512], dt.float32, tag=f"den{i}",
                                     name=f"den{i}")
                    nc.vector.tensor_copy(den[i][:], ot_ps[DH:DH + 1, :])
                rec = {}
                for i in (0, 1):
                    rec[i] = pN.tile([1, 512], dt.float32, tag=f"rec{i}",
                                     name=f"rec{i}")
                    nc.vector.reciprocal_approx_fast(rec[i][:], den[i][:])
                for i in (0, 1):
                    rech[i] = pN.tile([1, 512], dt.float16, tag=f"rech{i}",
                                      name=f"rech{i}")
                    nc.vector.tensor_copy(rech[i][:], rec[i][:])

                def stage2(i, ot_ps):
                    # broadcast across 64 partitions + apply
                    bc = psX.tile([64, 512], dt.float32, tag="psX")
                    nc.tensor.matmul(bc[:], sel_sb[0:1, 0:64], rech[i][:],
                                     start=True, stop=True)
                    bcs = pN.tile([64, 512], dt.float32, tag=f"bcs{i}")
                    nc.vector.tensor_copy(bcs[:], bc[:])
                    nc.vector.tensor_mul(
                        ot_f[pr][i * 64:(i + 1) * 64, qs],
                        ot_ps[0:DH, :], bcs[:])
                return (lambda: stage2(0, ot_e)), (lambda: stage2(1, ot_o))

            def out_proj_tile(tt):
                ps = psX.tile([128, 512], dt.float32, tag="psX")
                for dc in range(DC):
                    nc.tensor.matmul(
                        ps[:],
                        ot_f[dc][:, tt * 128:(tt + 1) * 128],
                        wo_sb[:, dc, :],
                        start=(dc == 0),
                        stop=(with_bias is False and dc == DC - 1))
                if with_bias:
                    nc.tensor.matmul(ps[:], ones1_sb[:],
                                     bvo_sb[:, 1, :],
                                     start=False, stop=True)
                os = pC.tile([128, 512], dt.float32, tag="os")
                nc.vector.tensor_copy(os[:], ps[:])
                nc.sync.dma_start(out[tt * 128:(tt + 1) * 128, :], os[:])

            # ---- fused attention rounds: jq outer, pr mid, kc inner.
            # PV lags scores by 2 rounds; the previous sweep's
            # normalization and the finished half's output projection are
            # deferred into the first rounds of the following sweep so
            # they never head-of-line-block the PE queue.
            deferred = []      # norm stage-2 closures, drained rounds 2-3
            def_op = []        # out-proj closures, drained rounds 8..14
            for jq in range(2):
                qs = slice(jq * 512, (jq + 1) * 512)
                for pr in range(H // 2):
                    need(("q", pr, jq))
                    ot_e = psO.tile([DH + 1, 512], dt.float32, tag="ote")
                    ot_o = psO.tile([DH + 1, 512], dt.float32, tag="oto")
                    pvq = []
                    for kc in range(KC):
                        need(("k", pr, kc // 4))
                        if pr == 0 and jq == 0:
                            if kc >= 2:
                                need(("v", kc - 2, 0))
                            if kc >= 16 and kc % 2 == 1:
                                drain_bg(1)
                        elif jq == 0 and kc % 2 == 0:
                            drain_bg(1)
                        ks = slice(kc * 128, (kc + 1) * 128)
                        sp = psS.tile([128, 1024], dt.float32, tag="sp")
                        nc.tensor.matmul(
                            sp[:, 0:512],
                            kt_f[pr][0:64, ks], qt_f[pr][0:64, qs],
                            start=True, stop=True)
                        nc.tensor.matmul(
                            sp[:, 512:1024],
                            kt_f[pr][64:128, ks], qt_f[pr][64:128, qs],
                            start=True, stop=True)
                        p_sb = pP.tile([128, 1024], dt.bfloat16, tag="p")
                        nc.scalar.activation(p_sb[:], sp[:], AF.Exp,
                                             scale=0.125)
                        mb = mt[(jq * 8 + kc // 4) % NMB]
                        pv = p_sb[:].rearrange("p (a b) -> p a b", a=2)
                        nc.vector.tensor_mul(
                            pv, pv,
                            mb[:, kc % 4, :][:, None, :]
                            .to_broadcast((128, 2, 512)))

                        def pv_mm(kc=kc, p_sb=p_sb):
                            nc.tensor.matmul(
                                ot_e[:], v_g[kc // 8][:, kc % 8, 2 * pr, :],
                                p_sb[:, 0:512],
                                start=(kc == 0), stop=(kc == KC - 1))
                            nc.tensor.matmul(
                                ot_o[:], v_g[kc // 8][:, kc % 8, 2 * pr + 1, :],
                                p_sb[:, 512:1024],
                                start=(kc == 0), stop=(kc == KC - 1))
                        pvq.append(pv_mm)
                        if deferred and kc in (2, 3):
                            deferred.pop(0)()
                        if def_op and kc >= 8 and kc % 2 == 0:
                            def_op.pop(0)()
                        if len(pvq) > (4 if kc < 8 else 2):
                            pvq.pop(0)()
                        if jq == 0 and pr == 3 and kc % 4 == 3 and kc // 4 >= 1:
                            # refill mask buffer with jq1's next group once
                            # its last jq0 reader (this round) is done
                            nc.sync.dma_start(mt[(8 + kc // 4) % NMB][:],
                                              msk[8 + kc // 4])
                    if pr == 0 and jq == 0:
                        need(("v", KC - 2, 0))
                        need(("v", KC - 1, 0))
                    while pvq:
                        pvq.pop(0)()
                    assert not deferred
                    # queue this sweep's normalization for the next sweep
                    s2e, s2o = norm_pair(ot_e, ot_o, pr, qs)
                    deferred.append(s2e)
                    deferred.append(s2o)
                    if pr == H // 2 - 1:
                        for tt in range(jq * 4, (jq + 1) * 4):
                            def_op.append(lambda tt=tt: out_proj_tile(tt))
            while deferred:
                deferred.pop(0)()
            while def_op:
                def_op.pop(0)()

    nc.compile()
    return nc


def _get_nc(with_bias: bool):
    if with_bias not in _BUILT:
        _BUILT[with_bias] = _build(with_bias)
    return _BUILT[with_bias]


def _prep_inputs(x, Wq, bq, Wk, bk, Wv, bv, Wo, bo, mask, with_bias):
    bf16 = ml_dtypes.bfloat16

    shared = {}
    for name, W in (("wq", Wq), ("wk", Wk), ("wv", Wv), ("wo", Wo)):
        shared[name] = np.ascontiguousarray(
            np.asarray(W, np.float32).astype(bf16)
            .reshape(DC, 128, D).transpose(1, 0, 2))
    selm = np.zeros((2, 128), np.float32)
    selm[0, 0:64] = 1.0
    selm[1, 64:128] = 1.0
    shared["sel"] = selm.astype(np.float16)
    if with_bias:
        shared["bqk"] = np.ascontiguousarray(np.stack(
            [np.asarray(bq, np.float32).reshape(DC, 128).T,
             np.asarray(bk, np.float32).reshape(DC, 128).T], axis=-1))
        shared["bvo"] = np.ascontiguousarray(np.stack(
            [np.asarray(bv, np.float32), np.asarray(bo, np.float32)]
        ).astype(bf16).reshape(1, 2, D))
        shared["ones1"] = np.ones((1, 128), np.float32).astype(bf16)

    maskT = np.asarray(mask).reshape(T, T).T          # (k, q)
    m01T = maskT.astype(np.float32)

    in_maps = []
    for c in range(N_CORES):
        b, qlo = c // 4, (c % 4) * QPC
        xTb = np.asarray(x[b], np.float32).T.astype(bf16)     # (D, T)
        m = dict(shared)
        m["xT"] = np.ascontiguousarray(
            xTb.reshape(DC, 128, T).transpose(1, 0, 2))
        m["xQT"] = np.ascontiguousarray(
            xTb[:, qlo:qlo + QPC].reshape(DC, 128, QPC).transpose(1, 0, 2))
        # (k, q) block of this core's queries -> [jq, g, 128, 4, 512]
        m01 = m01T[:, qlo:qlo + QPC].reshape(8, 4, 128, 2, 512)
        m["msk"] = np.ascontiguousarray(
            m01.transpose(3, 0, 2, 1, 4).reshape(16, 128, 4, 512)).astype(bf16)
        in_maps.append(m)
    return in_maps


def _run(inputs, trace=False):
    from concourse.bass_utils import run_bass_kernel_spmd

    with_bias = any(
        float(np.abs(np.asarray(inputs[k], np.float32)).max()) != 0.0
        for k in ("bq", "bk", "bv", "bo"))
    nc = _get_nc(with_bias)
    in_maps = _prep_inputs(
        inputs["x"], inputs["Wq"], inputs["bq"], inputs["Wk"], inputs["bk"],
        inputs["Wv"], inputs["bv"], inputs["Wo"], inputs["bo"],
        inputs["mask"], with_bias)
    res = run_bass_kernel_spmd(nc, in_maps, list(range(N_CORES)), trace=trace)
    O = np.empty((B, T, D), np.float32)
    for c in range(N_CORES):
        b, qlo = c // 4, (c % 4) * QPC
        O[b, qlo:qlo + QPC, :] = res.results[c]["out"]
    return O, res


def kernel(**inputs) -> np.ndarray:
    out, _ = _run(inputs, trace=False)
    return out
